# revision 25
# baseline (speedup 1.0000x reference)
"""
Trainium2 Bass kernel for nn_Attention_335007449901 (sparse window attention).

Model (per image, eval mode):
  q = BN(conv1x1(x, wq)); k = BN(conv1x1(x, wk)); v = BN(conv1x1(x, wv))
  7x7 windows over the 112x112 image -> T=256 window tokens, token
  features = (channel, within-window position p) pairs.
  dots[i,j] = <q_i, k_j> * 0.125 ; attn = softmax_j ; out = attn @ v
  y = gelu(out); z = BN(conv1x1(y, wo) + bo); out = gelu(z + x)

Sharding: pure data parallel over batch, 4 images per core on 8 cores.

Implementation notes:
  * BatchNorms folded into conv weights on the host; SCALE folded into q's
    path; k's bias drops (softmax shift invariance along the normalized
    axis); v's bias passes through the attention average (rows sum to 1)
    into the first gelu's bias; the final conv bias + BN fold into the last
    gelu's bias.
  * q and k never materialize: dots_T[j,i] = sum_p x_pj^T M x_pi with
    M = wk_f^T wq_f precomputed on the host, computed as u_p = M^T x_p
    then dots_T += u_p^T x_p. q's bias contributes a per-row term
    c[j] = sum_p (wk_f^T Bq) . x_p[:,j], accumulated with rank-reduce
    matmuls and applied as the per-partition bias of the softmax exp.
  * All matmul operands are bf16 (fp32 PSUM accumulation): fp32 matmuls on
    trn2 run as LOW/HIGH double passes, and strided moving operands stream
    ~5x slower, so a window-permuted contiguous bf16 copy of x (x_winb,
    built by the otherwise idle GPSIMD engine) feeds every matmul.
  * dots are computed transposed so softmax normalization is a ones-vector
    matmul reduce; no max subtraction needed (|dots| < ~30, fp32 exp safe).
  * The residual add reads the original fp32 x image; the final gelu writes
    its output IN PLACE into the x image (each window position's columns
    are dead after their residual read), saving a whole image buffer.
"""

import numpy as np

IN_C = 128
HIDE_C = 256
HC2 = 128
OUT_C = 128
WS = 7
SCALE = 0.125
EPS = 1e-5
B, H, W = 32, 112, 112
HW = H * W          # 12544
H1 = H // WS        # 16
W1 = W // WS        # 16
T = H1 * W1         # 256 windows
NP = WS * WS        # 49 positions
NCORES = 8
BPC = B // NCORES   # images per core

F32 = np.float32


def _pgroups():
    """Groups of 1-2 within-window positions with a uniform pixel-offset
    stride between members (one strided access pattern per group in the
    image layout). 49 positions -> 24 pairs + 1 singleton. g[2] is the
    group's column base in the position-major window layout x_winb."""
    groups = []
    base = 0
    for ws1 in range(WS):
        for b2 in range(3):
            groups.append(((ws1, 2 * b2), (ws1, 2 * b2 + 1), base))
            base += 2 * T
    for a in range(3):
        groups.append(((2 * a, 6), (2 * a + 1, 6), base))
        base += 2 * T
    groups.append(((6, 6), None, base))
    return groups


def build_bass_kernel(bpc=BPC):
    import concourse.bass as bass
    import concourse.tile as tile
    import concourse.mybir as mybir
    from concourse import bacc

    f32 = mybir.dt.float32
    bf16 = mybir.dt.bfloat16
    AF = mybir.ActivationFunctionType

    nc = bacc.Bacc("TRN2", target_bir_lowering=False)

    x_d = nc.dram_tensor("x", [bpc, IN_C, HW], f32, kind="ExternalInput")
    m_d = nc.dram_tensor("m", [IN_C, IN_C], bf16, kind="ExternalInput")
    h_d = nc.dram_tensor("hcol", [IN_C, 1], bf16, kind="ExternalInput")
    wvT_d = nc.dram_tensor("wvT", [IN_C, HIDE_C], bf16, kind="ExternalInput")
    woT_d = nc.dram_tensor("woT", [HIDE_C, OUT_C], bf16, kind="ExternalInput")
    # packed per-partition fp32 bias columns: [Bv_lo, Bv_hi, Bo]
    bias_d = nc.dram_tensor("biases", [128, 3], f32, kind="ExternalInput")
    out_d = nc.dram_tensor("out", [bpc, OUT_C, HW], f32, kind="ExternalOutput")

    groups = _pgroups()

    with tile.TileContext(nc) as tc:
        with (
            tc.tile_pool(name="singles", bufs=1) as singles,
            tc.tile_pool(name="xpool", bufs=2) as xpool,
            tc.tile_pool(name="xwin", bufs=2) as xwin_pool,
            tc.tile_pool(name="u_sb", bufs=4) as u_sb_pool,
            tc.tile_pool(name="v_sb", bufs=12) as v_sb_pool,
            tc.tile_pool(name="g_sb", bufs=2) as g_sb_pool,
            tc.tile_pool(name="attn_sb", bufs=2) as attn_pool,
            tc.tile_pool(name="tmp_sb", bufs=2) as tmp_pool,
            tc.tile_pool(name="small_sb", bufs=2) as small_pool,
            tc.tile_pool(name="ps_work", bufs=2, space="PSUM") as ps_work,
            tc.tile_pool(name="ps_dots", bufs=1, space="PSUM") as ps_dots,
            tc.tile_pool(name="ps_c", bufs=1, space="PSUM") as ps_c,
            tc.tile_pool(name="ps_av", bufs=2, space="PSUM") as ps_av,
            tc.tile_pool(name="ps_o", bufs=2, space="PSUM") as ps_o_pool,
        ):
            # ---- weights / constants (resident) ----
            m_sb = singles.tile([128, IN_C], bf16)
            nc.sync.dma_start(out=m_sb, in_=m_d.ap())
            h_sb = singles.tile([128, 1], bf16)
            nc.sync.dma_start(out=h_sb, in_=h_d.ap())
            wvT = singles.tile([128, HIDE_C], bf16)
            nc.sync.dma_start(out=wvT, in_=wvT_d.ap())
            woT = singles.tile([128, 2, OUT_C], bf16)
            nc.sync.dma_start(
                out=woT, in_=woT_d.ap().rearrange("(kc p) m -> p kc m", kc=2)
            )
            biases = singles.tile([128, 3], f32)
            nc.sync.dma_start(out=biases, in_=bias_d.ap())
            bv_ap = [biases[:, 0:1], biases[:, 1:2]]
            bo_ap = biases[:, 2:3]

            ones_mat = singles.tile([128, 128], bf16)
            nc.vector.memset(ones_mat, 1.0)
            ones_row = singles.tile([1, T], bf16)
            nc.vector.memset(ones_row, 1.0)

            for img in range(bpc):
                # ---- load x image; build position-major bf16 window copy ----
                x_img = xpool.tile([128, HW], f32, tag="ximg")
                for dc in range(4):
                    nc.sync.dma_start(
                        out=x_img[:, dc * (HW // 4):(dc + 1) * (HW // 4)],
                        in_=x_d.ap()[img, :, dc * (HW // 4):(dc + 1) * (HW // 4)])
                # dummy exp so walrus places the exp ACT-table load here,
                # off the softmax critical chain
                scratch = small_pool.tile([128, 1], f32, tag="scratch")
                nc.scalar.activation(scratch, biases[:, 0:1], AF.Exp)
                x5 = x_img.rearrange("p (h a w b) -> p h a w b", h=H1, a=WS, b=WS)

                def grp_ap(g):
                    """strided image-layout AP of this group's positions"""
                    (ws1, ws2), p2, _ = g
                    if p2 is None:
                        return x5[:, :, ws1, :, ws2]
                    if p2[0] == ws1:  # within-row pair, pixel stride 1
                        return x5[:, :, ws1, :, ws2:ws2 + 2].rearrange(
                            "p h w b -> p b h w")
                    return x5[:, :, ws1:ws1 + 2, :, ws2].rearrange(
                        "p h a w -> p a h w")

                x_winb = xwin_pool.tile([128, NP * T], bf16, tag="xwin")
                for gi, g in enumerate(groups):
                    N = T if g[1] is None else 2 * T
                    dst = x_winb[:, g[2]:g[2] + N]
                    # split the window permute across the three copy engines
                    if gi < 13:
                        nc.gpsimd.tensor_copy(dst, grp_ap(g))
                    elif gi % 2 == 0:
                        nc.scalar.activation(dst, grp_ap(g), AF.Copy, scale=1.0)
                    else:
                        nc.vector.tensor_copy(dst, grp_ap(g))

                # ---- phase 1: dots_T and c accumulation over positions ----
                dots_t = ps_dots.tile([128, 512], f32, tag="dots", name="dots")
                dots = [dots_t[:, 0:T], dots_t[:, T:2 * T]]
                c_row_ps = ps_c.tile([1, T], f32, tag="cps", name="cps")
                chunk_starts = list(range(0, NP, 2))   # 2 positions per chunk
                nchunks = len(chunk_starts)

                def u_conv(ci, p0):
                    npos = min(2, NP - p0)
                    N = npos * T
                    base = p0 * T
                    u_ps = ps_work.tile([128, 512], f32, tag="pwork")
                    nc.tensor.matmul(u_ps[:, :N], lhsT=m_sb,
                                     rhs=x_winb[:, base:base + N],
                                     start=True, stop=True)
                    u_sbt = u_sb_pool.tile([128, 512], bf16, tag="u")
                    if ci % 2 == 0:
                        nc.scalar.activation(u_sbt[:, :N], u_ps[:, :N],
                                             AF.Copy, scale=1.0)
                    else:
                        nc.vector.tensor_copy(u_sbt[:, :N], u_ps[:, :N])
                    return u_sbt

                def dots_mms(ci, p0, u_sbt):
                    npos = min(2, NP - p0)
                    base = p0 * T
                    first = ci == 0
                    for pi in range(npos):
                        for jh in (0, 1):
                            nc.tensor.matmul(
                                dots[jh],
                                lhsT=u_sbt[:, pi * T + jh * 128:
                                           pi * T + jh * 128 + 128],
                                rhs=x_winb[:, base + pi * T:
                                           base + (pi + 1) * T],
                                start=first and pi == 0 and jh == 0,
                                stop=False,
                                skip_group_check=True)

                pend = []
                for ci, p0 in enumerate(chunk_starts):
                    u_sbt = u_conv(ci, p0)
                    if len(pend) >= 2:
                        dots_mms(*pend.pop(0))
                    pend.append((ci, p0, u_sbt))
                    if ci == 12:
                        # first half of the c[j] run (x_winb cols ready);
                        # h stays loaded across the run
                        for p in range(24):
                            nc.tensor.matmul(c_row_ps, lhsT=h_sb,
                                             rhs=x_winb[:, p * T:(p + 1) * T],
                                             start=p == 0, stop=False)
                for pe_ in pend:
                    dots_mms(*pe_)
                # c[j] = sum_p h . x_p[:, j], then added into dots via two
                # rank-1 matmuls
                for p in range(24, NP):
                    nc.tensor.matmul(c_row_ps, lhsT=h_sb,
                                     rhs=x_winb[:, p * T:(p + 1) * T],
                                     start=False, stop=p == NP - 1)
                c_row = small_pool.tile([1, T], bf16, tag="csb")
                nc.scalar.activation(c_row, c_row_ps, AF.Copy, scale=1.0)
                for jh in (0, 1):
                    nc.tensor.matmul(
                        dots[jh], lhsT=c_row[:, jh * 128:jh * 128 + 128],
                        rhs=ones_row, start=False, stop=jh == 1,
                        skip_group_check=True)

                # ---- early v-convs (overlap the softmax chain on PE) ----
                def v_conv(g):
                    cnt = 1 if g[1] is None else 2
                    vsb = []
                    for pi in range(cnt):
                        v_ps = ps_work.tile([128, 512], f32, tag="pwork")
                        for jc in (0, 1):
                            nc.tensor.matmul(
                                v_ps[:, jc * HIDE_C:(jc + 1) * HIDE_C],
                                lhsT=x_winb[:, g[2] + pi * T + jc * 128:
                                            g[2] + pi * T + jc * 128 + 128],
                                rhs=wvT,
                                start=True, stop=True)
                        v_sbt = v_sb_pool.tile([128, 512], bf16, tag="v")
                        nc.vector.tensor_copy(v_sbt, v_ps)
                        vsb.append(v_sbt)
                    return vsb

                NEARLY = 6
                early_v = [v_conv(g) for g in groups[:NEARLY]]

                # ---- softmax over j (= partitions of dots_T) ----
                attn = [attn_pool.tile([128, T], bf16, tag=f"attn{jc}",
                                       name=f"attn{jc}") for jc in (0, 1)]
                for jc in (0, 1):
                    nc.scalar.activation(attn[jc], dots[jc], AF.Exp)
                s_ps = ps_dots.tile([128, T], f32, tag="dots", name="ssum")
                for jc in (0, 1):
                    nc.tensor.matmul(s_ps, lhsT=ones_mat, rhs=attn[jc],
                                     start=jc == 0, stop=jc == 1)
                r_sb = small_pool.tile([128, T], f32, tag="rsb")
                nc.vector.reciprocal(r_sb, s_ps)
                for jc in (0, 1):
                    nc.vector.tensor_mul(attn[jc], attn[jc], r_sb)

                # ---- phase 2: attention-average, out-conv, residual ----
                vcache = dict(enumerate(early_v))
                for gi, g in enumerate(groups):
                    cnt = 1 if g[1] is None else 2
                    N = cnt * T
                    if gi + NEARLY < len(groups):
                        vcache[gi + NEARLY] = v_conv(groups[gi + NEARLY])
                    vsb = vcache.pop(gi)

                    g_tiles = []
                    for kc in (0, 1):
                        av = ps_av.tile([128, 512], f32, tag="av", name=f"av{kc}")
                        for pi in range(cnt):
                            for jc in (0, 1):
                                nc.tensor.matmul(
                                    av[:, pi * T:(pi + 1) * T],
                                    lhsT=vsb[pi][:, jc * HIDE_C + kc * 128:
                                                  jc * HIDE_C + kc * 128 + 128],
                                    rhs=attn[jc],
                                    start=jc == 0, stop=jc == 1)
                        g_t = g_sb_pool.tile([128, 512], bf16, tag=f"g{kc}")
                        nc.scalar.activation(g_t[:, :N], av[:, :N], AF.Gelu,
                                             bias=bv_ap[kc], scale=1.0)
                        g_tiles.append(g_t)

                    o_ps = ps_o_pool.tile([128, 512], f32, tag="ops")
                    for pi in range(cnt):
                        for kc in (0, 1):
                            nc.tensor.matmul(
                                o_ps[:, pi * T:(pi + 1) * T],
                                lhsT=woT[:, kc, :],
                                rhs=g_tiles[kc][:, pi * T:(pi + 1) * T],
                                start=kc == 0, stop=kc == 1)
                    # residual add (fp32 x) + final gelu, written back IN
                    # PLACE into the x image (columns dead after the read)
                    tmp = tmp_pool.tile([128, 512], f32, tag="tmp")
                    nc.vector.tensor_add(tmp[:, :N], o_ps[:, :N], grp_ap(g))
                    nc.scalar.activation(grp_ap(g), tmp[:, :N], AF.Gelu,
                                         bias=bo_ap, scale=1.0)

                # ---- store (x_img now holds the output image) ----
                for dc in range(4):
                    nc.sync.dma_start(
                        out=out_d.ap()[img, :, dc * (HW // 4):(dc + 1) * (HW // 4)],
                        in_=x_img[:, dc * (HW // 4):(dc + 1) * (HW // 4)])

    nc.compile()
    return nc


def fold_params(wq, gq, bq, mq, vq, wk, gk, bk, mk, vk,
                wv, gv, bv, mv, vv, wo, bo, go, bbo, mo, vo):
    """Host-side BN/bias folding. Returns (M, h, wvT, woT, biases)."""
    import ml_dtypes
    bf16 = ml_dtypes.bfloat16

    aq = gq / np.sqrt(vq + EPS)
    wq_f = (SCALE * aq)[:, None] * wq
    Bq = SCALE * (bq - aq * mq)

    ak = gk / np.sqrt(vk + EPS)
    wk_f = ak[:, None] * wk          # k bias drops (softmax shift invariance)

    M = wk_f.T @ wq_f                # dots_T = sum_p (M^T x_p)^T x_p
    hv = wk_f.T @ Bq                 # c[j] = sum_p hv . x_p[:, j]

    av = gv / np.sqrt(vv + EPS)
    wv_f = av[:, None] * wv
    Bv = bv - av * mv                # applied inside the first gelu

    ao = go / np.sqrt(vo + EPS)
    wo_f = ao[:, None] * wo
    Bo = ao * (bo - mo) + bbo        # conv bias + BN fold, inside last gelu

    biases = np.stack([Bv[:128], Bv[128:], Bo], axis=1).astype(F32)
    return (np.ascontiguousarray(M).astype(bf16),
            np.ascontiguousarray(hv[:, None]).astype(bf16),
            np.ascontiguousarray(wv_f.T).astype(bf16),
            np.ascontiguousarray(wo_f.T).astype(bf16),
            biases)


_CACHED = {}


def _get_nc(bpc=BPC):
    if bpc not in _CACHED:
        _CACHED[bpc] = build_bass_kernel(bpc)
    return _CACHED[bpc]


def make_in_maps(inputs):
    x = np.asarray(inputs["x"], F32)
    m, hv, wvT, woT, biases = fold_params(
        *[np.asarray(inputs[k], F32) for k in
          ("wq", "gq", "bq", "mq", "vq", "wk", "gk", "bk", "mk", "vk",
           "wv", "gv", "bv", "mv", "vv", "wo", "bo", "go", "bbo", "mo", "vo")]
    )
    in_maps = []
    for c in range(NCORES):
        xs = np.ascontiguousarray(
            x[c * BPC:(c + 1) * BPC].reshape(BPC, IN_C, HW))
        in_maps.append({"x": xs, "m": m, "hcol": hv, "wvT": wvT,
                        "woT": woT, "biases": biases})
    return in_maps


def kernel(**inputs):
    from concourse.bass_utils import run_bass_kernel_spmd

    in_maps = make_in_maps(inputs)
    nc = _get_nc(BPC)
    res = run_bass_kernel_spmd(nc, in_maps, list(range(NCORES)))
    outs = [res.results[c]["out"].reshape(BPC, OUT_C, H, W)
            for c in range(NCORES)]
    return np.concatenate(outs, axis=0)


# revision 26
# speedup vs baseline: 1.0567x; 1.0567x over previous
"""
Trainium2 Bass kernel for nn_Attention_335007449901 (sparse window attention).

Model (per image, eval mode):
  q = BN(conv1x1(x, wq)); k = BN(conv1x1(x, wk)); v = BN(conv1x1(x, wv))
  7x7 windows over the 112x112 image -> T=256 window tokens, token
  features = (channel, within-window position p) pairs.
  dots[i,j] = <q_i, k_j> * 0.125 ; attn = softmax_j ; out = attn @ v
  y = gelu(out); z = BN(conv1x1(y, wo) + bo); out = gelu(z + x)

Sharding: pure data parallel over batch, 4 images per core on 8 cores.

Implementation notes:
  * BatchNorms folded into conv weights on the host; SCALE folded into q's
    path; k's bias drops (softmax shift invariance along the normalized
    axis); v's bias passes through the attention average (rows sum to 1)
    into the first gelu's bias; the final conv bias + BN fold into the last
    gelu's bias.
  * q and k never materialize: dots_T[j,i] = sum_p x_pj^T M x_pi with
    M = wk_f^T wq_f precomputed on the host, computed as u_p = M^T x_p
    then dots_T += u_p^T x_p. q's bias contributes a per-row term
    c[j] = sum_p (wk_f^T Bq) . x_p[:,j], accumulated with rank-reduce
    matmuls and applied as the per-partition bias of the softmax exp.
  * All matmul operands are bf16 (fp32 PSUM accumulation): fp32 matmuls on
    trn2 run as LOW/HIGH double passes, and strided moving operands stream
    ~5x slower, so a window-permuted contiguous bf16 copy of x (x_winb,
    built by the otherwise idle GPSIMD engine) feeds every matmul.
  * dots are computed transposed so softmax normalization is a ones-vector
    matmul reduce; no max subtraction needed (|dots| < ~30, fp32 exp safe).
  * The residual add reads the original fp32 x image; the final gelu writes
    its output IN PLACE into the x image (each window position's columns
    are dead after their residual read), saving a whole image buffer.
"""

import numpy as np

IN_C = 128
HIDE_C = 256
HC2 = 128
OUT_C = 128
WS = 7
SCALE = 0.125
EPS = 1e-5
B, H, W = 32, 112, 112
HW = H * W          # 12544
H1 = H // WS        # 16
W1 = W // WS        # 16
T = H1 * W1         # 256 windows
NP = WS * WS        # 49 positions
NCORES = 8
BPC = B // NCORES   # images per core

F32 = np.float32


def _pgroups():
    """Groups of 1-2 within-window positions with a uniform pixel-offset
    stride between members (one strided access pattern per group in the
    image layout). 49 positions -> 24 pairs + 1 singleton. g[2] is the
    group's column base in the position-major window layout x_winb."""
    groups = []
    base = 0
    for ws1 in range(WS):
        for b2 in range(3):
            groups.append(((ws1, 2 * b2), (ws1, 2 * b2 + 1), base))
            base += 2 * T
    for a in range(3):
        groups.append(((2 * a, 6), (2 * a + 1, 6), base))
        base += 2 * T
    groups.append(((6, 6), None, base))
    return groups


def build_bass_kernel(bpc=BPC):
    import concourse.bass as bass
    import concourse.tile as tile
    import concourse.mybir as mybir
    from concourse import bacc

    f32 = mybir.dt.float32
    bf16 = mybir.dt.bfloat16
    AF = mybir.ActivationFunctionType

    nc = bacc.Bacc("TRN2", target_bir_lowering=False)

    x_d = nc.dram_tensor("x", [bpc, IN_C, HW], f32, kind="ExternalInput")
    m_d = nc.dram_tensor("m", [IN_C, IN_C], bf16, kind="ExternalInput")
    h_d = nc.dram_tensor("hcol", [IN_C, 1], bf16, kind="ExternalInput")
    wvT_d = nc.dram_tensor("wvT", [IN_C, HIDE_C], bf16, kind="ExternalInput")
    woT_d = nc.dram_tensor("woT", [HIDE_C, OUT_C], bf16, kind="ExternalInput")
    # packed per-partition fp32 bias columns: [Bv_lo, Bv_hi, Bo]
    bias_d = nc.dram_tensor("biases", [128, 3], f32, kind="ExternalInput")
    out_d = nc.dram_tensor("out", [bpc, OUT_C, HW], f32, kind="ExternalOutput")

    groups = _pgroups()

    with tile.TileContext(nc) as tc:
        with (
            tc.tile_pool(name="singles", bufs=1) as singles,
            tc.tile_pool(name="xpool", bufs=2) as xpool,
            tc.tile_pool(name="xwin", bufs=2) as xwin_pool,
            tc.tile_pool(name="u_sb", bufs=4) as u_sb_pool,
            tc.tile_pool(name="v_sb", bufs=12) as v_sb_pool,
            tc.tile_pool(name="g_sb", bufs=2) as g_sb_pool,
            tc.tile_pool(name="attn_sb", bufs=2) as attn_pool,
            tc.tile_pool(name="tmp_sb", bufs=2) as tmp_pool,
            tc.tile_pool(name="small_sb", bufs=2) as small_pool,
            tc.tile_pool(name="ps_work", bufs=2, space="PSUM") as ps_work,
            tc.tile_pool(name="ps_dots", bufs=1, space="PSUM") as ps_dots,
            tc.tile_pool(name="ps_c", bufs=1, space="PSUM") as ps_c,
            tc.tile_pool(name="ps_av", bufs=2, space="PSUM") as ps_av,
            tc.tile_pool(name="ps_o", bufs=2, space="PSUM") as ps_o_pool,
        ):
            # ---- weights / constants (resident) ----
            m_sb = singles.tile([128, IN_C], bf16)
            nc.sync.dma_start(out=m_sb, in_=m_d.ap())
            h_sb = singles.tile([128, 1], bf16)
            nc.sync.dma_start(out=h_sb, in_=h_d.ap())
            wvT = singles.tile([128, HIDE_C], bf16)
            nc.sync.dma_start(out=wvT, in_=wvT_d.ap())
            woT = singles.tile([128, 2, OUT_C], bf16)
            nc.sync.dma_start(
                out=woT, in_=woT_d.ap().rearrange("(kc p) m -> p kc m", kc=2)
            )
            biases = singles.tile([128, 3], f32)
            nc.sync.dma_start(out=biases, in_=bias_d.ap())
            bv_ap = [biases[:, 0:1], biases[:, 1:2]]
            bo_ap = biases[:, 2:3]

            ones_mat = singles.tile([128, 128], bf16)
            nc.vector.memset(ones_mat, 1.0)
            ones_row = singles.tile([1, T], bf16)
            nc.vector.memset(ones_row, 1.0)

            for img in range(bpc):
                # ---- load x image; build position-major bf16 window copy ----
                x_img = xpool.tile([128, HW], f32, tag="ximg")
                for dc in range(4):
                    nc.sync.dma_start(
                        out=x_img[:, dc * (HW // 4):(dc + 1) * (HW // 4)],
                        in_=x_d.ap()[img, :, dc * (HW // 4):(dc + 1) * (HW // 4)])
                # dummy exp so walrus places the exp ACT-table load here,
                # off the softmax critical chain
                scratch = small_pool.tile([128, 1], f32, tag="scratch")
                nc.scalar.activation(scratch, biases[:, 0:1], AF.Exp)
                x5 = x_img.rearrange("p (h a w b) -> p h a w b", h=H1, a=WS, b=WS)

                def grp_ap(g):
                    """strided image-layout AP of this group's positions"""
                    (ws1, ws2), p2, _ = g
                    if p2 is None:
                        return x5[:, :, ws1, :, ws2]
                    if p2[0] == ws1:  # within-row pair, pixel stride 1
                        return x5[:, :, ws1, :, ws2:ws2 + 2].rearrange(
                            "p h w b -> p b h w")
                    return x5[:, :, ws1:ws1 + 2, :, ws2].rearrange(
                        "p h a w -> p a h w")

                x_winb = xwin_pool.tile([128, NP * T], bf16, tag="xwin")
                for gi, g in enumerate(groups):
                    N = T if g[1] is None else 2 * T
                    dst = x_winb[:, g[2]:g[2] + N]
                    # split the window permute across the three copy engines
                    if gi < 13:
                        nc.gpsimd.tensor_copy(dst, grp_ap(g))
                    elif gi % 2 == 0:
                        nc.scalar.activation(dst, grp_ap(g), AF.Copy, scale=1.0)
                    else:
                        nc.vector.tensor_copy(dst, grp_ap(g))

                # ---- phase 1: dots_T and c accumulation over positions ----
                dots_t = ps_dots.tile([128, 512], f32, tag="dots", name="dots")
                dots = [dots_t[:, 0:T], dots_t[:, T:2 * T]]
                c_row_ps = ps_c.tile([1, T], f32, tag="cps", name="cps")
                chunk_starts = list(range(0, NP, 2))   # 2 positions per chunk
                nchunks = len(chunk_starts)

                def u_conv(ci, p0):
                    npos = min(2, NP - p0)
                    N = npos * T
                    base = p0 * T
                    u_ps = ps_work.tile([128, 512], f32, tag="pwork")
                    nc.tensor.matmul(u_ps[:, :N], lhsT=m_sb,
                                     rhs=x_winb[:, base:base + N],
                                     start=True, stop=True)
                    u_sbt = u_sb_pool.tile([128, 512], bf16, tag="u")
                    if ci % 2 == 0:
                        nc.scalar.activation(u_sbt[:, :N], u_ps[:, :N],
                                             AF.Copy, scale=1.0)
                    else:
                        nc.vector.tensor_copy(u_sbt[:, :N], u_ps[:, :N])
                    return u_sbt

                def dots_mms(ci, p0, u_sbt):
                    npos = min(2, NP - p0)
                    base = p0 * T
                    first = ci == 0
                    for pi in range(npos):
                        for jh in (0, 1):
                            nc.tensor.matmul(
                                dots[jh],
                                lhsT=u_sbt[:, pi * T + jh * 128:
                                           pi * T + jh * 128 + 128],
                                rhs=x_winb[:, base + pi * T:
                                           base + (pi + 1) * T],
                                start=first and pi == 0 and jh == 0,
                                stop=False,
                                skip_group_check=True)

                pend = []
                for ci, p0 in enumerate(chunk_starts):
                    u_sbt = u_conv(ci, p0)
                    if len(pend) >= 2:
                        dots_mms(*pend.pop(0))
                    pend.append((ci, p0, u_sbt))
                    if ci == 12:
                        # first half of the c[j] run (x_winb cols ready);
                        # h stays loaded across the run
                        for p in range(24):
                            nc.tensor.matmul(c_row_ps, lhsT=h_sb,
                                             rhs=x_winb[:, p * T:(p + 1) * T],
                                             start=p == 0, stop=False)
                for pe_ in pend:
                    dots_mms(*pe_)
                # c[j] = sum_p h . x_p[:, j], then added into dots via two
                # rank-1 matmuls
                for p in range(24, NP):
                    nc.tensor.matmul(c_row_ps, lhsT=h_sb,
                                     rhs=x_winb[:, p * T:(p + 1) * T],
                                     start=False, stop=p == NP - 1)
                c_row = small_pool.tile([1, T], bf16, tag="csb")
                nc.vector.tensor_copy(c_row, c_row_ps)
                for jh in (0, 1):
                    nc.tensor.matmul(
                        dots[jh], lhsT=c_row[:, jh * 128:jh * 128 + 128],
                        rhs=ones_row, start=False, stop=jh == 1,
                        skip_group_check=True)

                # ---- early v-convs (overlap the softmax chain on PE) ----
                def v_conv(g):
                    cnt = 1 if g[1] is None else 2
                    vsb = []
                    for pi in range(cnt):
                        v_ps = ps_work.tile([128, 512], f32, tag="pwork")
                        for jc in (0, 1):
                            nc.tensor.matmul(
                                v_ps[:, jc * HIDE_C:(jc + 1) * HIDE_C],
                                lhsT=x_winb[:, g[2] + pi * T + jc * 128:
                                            g[2] + pi * T + jc * 128 + 128],
                                rhs=wvT,
                                start=True, stop=True)
                        v_sbt = v_sb_pool.tile([128, 512], bf16, tag="v")
                        nc.vector.tensor_copy(v_sbt, v_ps)
                        vsb.append(v_sbt)
                    return vsb

                NEARLY = 6
                early_v = [v_conv(g) for g in groups[:NEARLY]]

                # ---- softmax over j (= partitions of dots_T) ----
                attn = [attn_pool.tile([128, T], bf16, tag=f"attn{jc}",
                                       name=f"attn{jc}") for jc in (0, 1)]
                for jc in (0, 1):
                    nc.scalar.activation(attn[jc], dots[jc], AF.Exp)
                s_ps = ps_dots.tile([128, T], f32, tag="dots", name="ssum")
                for jc in (0, 1):
                    nc.tensor.matmul(s_ps, lhsT=ones_mat, rhs=attn[jc],
                                     start=jc == 0, stop=jc == 1)
                r_sb = small_pool.tile([128, T], f32, tag="rsb")
                nc.vector.reciprocal(r_sb, s_ps)
                for jc in (0, 1):
                    nc.vector.tensor_mul(attn[jc], attn[jc], r_sb)

                # ---- phase 2: attention-average, out-conv, residual ----
                vcache = dict(enumerate(early_v))
                for gi, g in enumerate(groups):
                    cnt = 1 if g[1] is None else 2
                    N = cnt * T
                    if gi + NEARLY < len(groups):
                        vcache[gi + NEARLY] = v_conv(groups[gi + NEARLY])
                    vsb = vcache.pop(gi)

                    g_tiles = []
                    for kc in (0, 1):
                        av = ps_av.tile([128, 512], f32, tag="av", name=f"av{kc}")
                        for pi in range(cnt):
                            for jc in (0, 1):
                                nc.tensor.matmul(
                                    av[:, pi * T:(pi + 1) * T],
                                    lhsT=vsb[pi][:, jc * HIDE_C + kc * 128:
                                                  jc * HIDE_C + kc * 128 + 128],
                                    rhs=attn[jc],
                                    start=jc == 0, stop=jc == 1)
                        g_t = g_sb_pool.tile([128, 512], bf16, tag=f"g{kc}")
                        nc.scalar.activation(g_t[:, :N], av[:, :N], AF.Gelu,
                                             bias=bv_ap[kc], scale=1.0)
                        g_tiles.append(g_t)

                    o_ps = ps_o_pool.tile([128, 512], f32, tag="ops")
                    for pi in range(cnt):
                        for kc in (0, 1):
                            nc.tensor.matmul(
                                o_ps[:, pi * T:(pi + 1) * T],
                                lhsT=woT[:, kc, :],
                                rhs=g_tiles[kc][:, pi * T:(pi + 1) * T],
                                start=kc == 0, stop=kc == 1)
                    # residual add (fp32 x) + final gelu, written back IN
                    # PLACE into the x image (columns dead after the read)
                    tmp = tmp_pool.tile([128, 512], f32, tag="tmp")
                    nc.vector.tensor_add(tmp[:, :N], o_ps[:, :N], grp_ap(g))
                    nc.scalar.activation(grp_ap(g), tmp[:, :N], AF.Gelu,
                                         bias=bo_ap, scale=1.0)

                # ---- store (x_img now holds the output image) ----
                for dc in range(4):
                    nc.sync.dma_start(
                        out=out_d.ap()[img, :, dc * (HW // 4):(dc + 1) * (HW // 4)],
                        in_=x_img[:, dc * (HW // 4):(dc + 1) * (HW // 4)])

    nc.compile()
    return nc


def fold_params(wq, gq, bq, mq, vq, wk, gk, bk, mk, vk,
                wv, gv, bv, mv, vv, wo, bo, go, bbo, mo, vo):
    """Host-side BN/bias folding. Returns (M, h, wvT, woT, biases)."""
    import ml_dtypes
    bf16 = ml_dtypes.bfloat16

    aq = gq / np.sqrt(vq + EPS)
    wq_f = (SCALE * aq)[:, None] * wq
    Bq = SCALE * (bq - aq * mq)

    ak = gk / np.sqrt(vk + EPS)
    wk_f = ak[:, None] * wk          # k bias drops (softmax shift invariance)

    M = wk_f.T @ wq_f                # dots_T = sum_p (M^T x_p)^T x_p
    hv = wk_f.T @ Bq                 # c[j] = sum_p hv . x_p[:, j]

    av = gv / np.sqrt(vv + EPS)
    wv_f = av[:, None] * wv
    Bv = bv - av * mv                # applied inside the first gelu

    ao = go / np.sqrt(vo + EPS)
    wo_f = ao[:, None] * wo
    Bo = ao * (bo - mo) + bbo        # conv bias + BN fold, inside last gelu

    biases = np.stack([Bv[:128], Bv[128:], Bo], axis=1).astype(F32)
    return (np.ascontiguousarray(M).astype(bf16),
            np.ascontiguousarray(hv[:, None]).astype(bf16),
            np.ascontiguousarray(wv_f.T).astype(bf16),
            np.ascontiguousarray(wo_f.T).astype(bf16),
            biases)


_CACHED = {}


def _get_nc(bpc=BPC):
    if bpc not in _CACHED:
        _CACHED[bpc] = build_bass_kernel(bpc)
    return _CACHED[bpc]


def make_in_maps(inputs):
    x = np.asarray(inputs["x"], F32)
    m, hv, wvT, woT, biases = fold_params(
        *[np.asarray(inputs[k], F32) for k in
          ("wq", "gq", "bq", "mq", "vq", "wk", "gk", "bk", "mk", "vk",
           "wv", "gv", "bv", "mv", "vv", "wo", "bo", "go", "bbo", "mo", "vo")]
    )
    in_maps = []
    for c in range(NCORES):
        xs = np.ascontiguousarray(
            x[c * BPC:(c + 1) * BPC].reshape(BPC, IN_C, HW))
        in_maps.append({"x": xs, "m": m, "hcol": hv, "wvT": wvT,
                        "woT": woT, "biases": biases})
    return in_maps


def kernel(**inputs):
    from concourse.bass_utils import run_bass_kernel_spmd

    in_maps = make_in_maps(inputs)
    nc = _get_nc(BPC)
    res = run_bass_kernel_spmd(nc, in_maps, list(range(NCORES)))
    outs = [res.results[c]["out"].reshape(BPC, OUT_C, H, W)
            for c in range(NCORES)]
    return np.concatenate(outs, axis=0)
